# revision 1
# baseline (speedup 1.0000x reference)
"""Trainium2 Bass kernel for GQA attention (B=4, T=2048, D=2048, 16 heads / 4 kv groups, RoPE).

Sharding: 8 cores = 4 batches x 2 head-halves. Core c handles batch c//2 and
heads (c%2)*8..+8 with kv groups (c%2)*2..+2.  Per core:
  qkvT projection (channel-major) with RoPE fused into the PSUM eviction,
  two-pass softmax (pass A: S[q,k] row-sums via activation accum_out;
  pass B: S^T[k,q] recomputed by swapping matmul operands, exp, PV in
  natural [q,d] orientation so 1/l is a per-partition scale at eviction),
  PE transpose to d-major, then row-parallel o_proj giving a partial
  [T, D] that the host sums across the two half cores of each batch.
All matmuls in bf16 with fp32 PSUM accumulation.
"""

import numpy as np
import ml_dtypes

BF16 = ml_dtypes.bfloat16

D_MODEL = 2048
NUM_HEADS = 16
QUERY_GROUPS = 4
HEAD_DIM = 128
B = 4
T = 2048
THETA = 10000.0
SCALE = 0.08838834764831845
N_CORES = 8

P = 128
NH = NUM_HEADS // 2          # 8 q heads per core
NG = QUERY_GROUPS // 2       # 2 kv groups per core
QDIM = NH * HEAD_DIM         # 1024
GDIM = NG * HEAD_DIM         # 256
NKT = D_MODEL // P           # 16 contraction tiles over d_model
NTT = T // P                 # 16 tiles over sequence
NCH = T // 512               # 4 chunks of 512 over sequence
NDT = QDIM // P              # 8 head/dim tiles per core


def build_nc(masked: bool):
    import concourse.bacc as bacc
    import concourse.tile as tile
    import concourse.mybir as mybir
    from concourse.masks import make_identity
    from contextlib import ExitStack

    dt = mybir.dt
    f32 = dt.float32
    bf16 = dt.bfloat16
    AF = mybir.ActivationFunctionType

    nc = bacc.Bacc("TRN2", target_bir_lowering=False, debug=False, num_devices=N_CORES)

    xt = nc.dram_tensor("xt", [D_MODEL, T], bf16, kind="ExternalInput")
    wqk = nc.dram_tensor("wqk", [D_MODEL, QDIM + GDIM], bf16, kind="ExternalInput")
    wv = nc.dram_tensor("wv", [D_MODEL, GDIM], bf16, kind="ExternalInput")
    wo = nc.dram_tensor("wo", [QDIM, D_MODEL], bf16, kind="ExternalInput")
    cosq = nc.dram_tensor("cosq", [P, T], f32, kind="ExternalInput")
    sinq = nc.dram_tensor("sinq", [P, T], f32, kind="ExternalInput")
    cosk = nc.dram_tensor("cosk", [P, T], f32, kind="ExternalInput")
    sink = nc.dram_tensor("sink", [P, T], f32, kind="ExternalInput")
    if masked:
        maskcol = nc.dram_tensor("maskcol", [P, NTT], f32, kind="ExternalInput")
    out = nc.dram_tensor("out", [T, D_MODEL], f32, kind="ExternalOutput")

    with tile.TileContext(nc) as tc:
        with ExitStack() as ctx:
            constp = ctx.enter_context(tc.tile_pool(name="const", bufs=1))
            qkT_pool = ctx.enter_context(tc.tile_pool(name="qkT", bufs=NH + NG))
            vnat_pool = ctx.enter_context(tc.tile_pool(name="vnat", bufs=NTT))

            identity = constp.tile([P, P], bf16, tag="identity")
            make_identity(nc, identity[:])
            if masked:
                maskcol_t = constp.tile([P, NTT], f32, tag="maskcol")
                nc.sync.dma_start(out=maskcol_t[:], in_=maskcol[:, :])

            # persistent bf16 tensors
            qkT = [qkT_pool.tile([P, T], bf16, tag="qkT", name=f"qkT{i}") for i in range(NH + NG)]
            # v_aug layout per t-tile: [v_g0 | ones | v_g1 | ones] so that the
            # 129-wide slice for group g is contiguous; the ones column makes
            # the PV matmul accumulate the softmax denominator in psum col 128.
            v_aug = [vnat_pool.tile([P, NG * (P + 1)], bf16, tag="vnat", name=f"vaug{i}")
                     for i in range(NTT)]
            for i in range(NTT):
                for g in range(NG):
                    nc.vector.memset(v_aug[i][:, g * (P + 1) + P:g * (P + 1) + P + 1], 1.0)

            # ---------------- phase 1: qkv projection + rope -------------
            with ExitStack() as ph1:
                tabp = ph1.enter_context(tc.tile_pool(name="tab", bufs=1))
                wqk_pool = ph1.enter_context(tc.tile_pool(name="wqk", bufs=NKT))
                wv_pool = ph1.enter_context(tc.tile_pool(name="wv", bufs=NKT))
                xc_pool = ph1.enter_context(tc.tile_pool(name="xc", bufs=NKT + 6))
                tmp_pool = ph1.enter_context(tc.tile_pool(name="rtmp", bufs=3))
                pj_pool = ph1.enter_context(
                    tc.tile_pool(name="pj", bufs=4, space="PSUM"))

                warm = pj_pool.tile([P, P], f32, tag="pj")
                for i in range(40):
                    nc.tensor.matmul(warm[:], lhsT=identity[:], rhs=identity[:],
                                     start=(i == 0), stop=(i == 39))
                wsink = tmp_pool.tile([P, 16], f32, tag="t1")
                nc.vector.tensor_copy(wsink[:], warm[:, 0:16])

                def load_xc(nch):
                    c0 = nch * 512
                    xc = []
                    for kt in range(NKT):
                        tl = xc_pool.tile([P, 512], bf16, tag="xc", name=f"xc{kt}")
                        nc.sync.dma_start(
                            out=tl[:], in_=xt[kt * P:(kt + 1) * P, c0:c0 + 512])
                        xc.append(tl)
                    return xc

                wv_t = []
                for kt in range(NKT):
                    tl = wv_pool.tile([P, GDIM], bf16, tag="wv", name=f"wvt{kt}")
                    nc.sync.dma_start(out=tl[:], in_=wv[kt * P:(kt + 1) * P, :])
                    wv_t.append(tl)
                xc_next = load_xc(0)
                wqk_t = []
                for kt in range(NKT):
                    tl = wqk_pool.tile([P, QDIM + GDIM], bf16, tag="wqk", name=f"wqkt{kt}")
                    nc.sync.dma_start(out=tl[:], in_=wqk[kt * P:(kt + 1) * P, :])
                    wqk_t.append(tl)
                tabs = {}
                for nm, tsrc in (("cosq", cosq), ("sinq", sinq),
                                 ("cosk", cosk), ("sink", sink)):
                    tl = tabp.tile([P, T], f32, tag=nm, name=nm + "_t")
                    nc.sync.dma_start(out=tl[:], in_=tsrc[:, :])
                    tabs[nm] = tl

                for nch in range(NCH):
                    c0 = nch * 512
                    xc = xc_next
                    if nch + 1 < NCH:
                        xc_next = load_xc(nch + 1)
                    # v projection first: depends only on xc + wv (2.5 MB),
                    # so PE starts before the full wqk lands
                    for tl_i in range(4):
                        tt = nch * 4 + tl_i
                        ps = pj_pool.tile([P, GDIM], f32, tag="pj")
                        for kt in range(NKT):
                            nc.tensor.matmul(
                                ps[:],
                                lhsT=xc[kt][:, tl_i * P:(tl_i + 1) * P],
                                rhs=wv_t[kt][:],
                                start=(kt == 0), stop=(kt == NKT - 1))
                        for g in range(NG):
                            nc.vector.tensor_copy(
                                v_aug[tt][:, g * (P + 1):g * (P + 1) + P],
                                ps[:, g * P:(g + 1) * P])
                    # q/k channel-major projection with fused rope eviction
                    for m in range(NH + NG):
                        ps = pj_pool.tile([P, 512], f32, tag="pj")
                        for kt in range(NKT):
                            nc.tensor.matmul(
                                ps[:],
                                lhsT=wqk_t[kt][:, m * P:(m + 1) * P],
                                rhs=xc[kt][:],
                                start=(kt == 0), stop=(kt == NKT - 1))
                        ct = tabs["cosq"] if m < NH else tabs["cosk"]
                        st = tabs["sinq"] if m < NH else tabs["sink"]
                        t1 = tmp_pool.tile([P, 512], f32, tag="t1")
                        t2 = tmp_pool.tile([P, 512], f32, tag="t2")
                        h = P // 2
                        nc.vector.tensor_mul(t1[:], ps[:], ct[:, c0:c0 + 512])
                        nc.vector.tensor_mul(
                            t2[0:h, :], ps[h:P, :], st[0:h, c0:c0 + 512])
                        nc.vector.tensor_mul(
                            t2[h:P, :], ps[0:h, :], st[h:P, c0:c0 + 512])
                        nc.vector.tensor_add(
                            qkT[m][:, c0:c0 + 512], t1[:], t2[:])

            # phase 2+ pools (opened after phase-1 pools free their SBUF/PSUM)
            mm_pool = ctx.enter_context(tc.tile_pool(name="mm", bufs=2, space="PSUM"))
            pv_pool = ctx.enter_context(tc.tile_pool(name="pv", bufs=4, space="PSUM"))
            attn_pool = ctx.enter_context(tc.tile_pool(name="attn", bufs=NTT))
            aT_pool = ctx.enter_context(tc.tile_pool(name="aT", bufs=NDT))
            rc_pool = ctx.enter_context(tc.tile_pool(name="rc", bufs=8))
            pt_pool = ctx.enter_context(tc.tile_pool(name="pt", bufs=4))
            osb_pool = ctx.enter_context(tc.tile_pool(name="osb", bufs=6))
            wo_pool = ctx.enter_context(tc.tile_pool(name="wo", bufs=NDT))
            attn_t = [attn_pool.tile([P, QDIM], bf16, tag="attn", name=f"attn{i}") for i in range(NTT)]
            aT = [aT_pool.tile([P, T], bf16, tag="aT", name=f"aT{i}") for i in range(NDT)]

            wo_t = []
            for dtile in range(NDT):
                tl = wo_pool.tile([P, D_MODEL], bf16, tag="wo", name=f"wot{dtile}")
                nc.sync.dma_start(out=tl[:], in_=wo[dtile * P:(dtile + 1) * P, :])
                wo_t.append(tl)

            # ---------------- phase 2+3: attention with interleaved o_proj ---
            # qc-outer / head-inner: after each q-chunk, the 4 finished
            # t-tiles are transposed and their o_proj runs, giving PE work
            # that overlaps the ACT-bound exp stretches of the next chunk.
            for qc in range(NCH):
                for h in range(NH):
                    g = h // 4  # local kv group
                    kTg = qkT[NH + g]
                    qTh = qkT[h]
                    pvs = [pv_pool.tile([P, P + 4], f32, tag="pv", name=f"pv{j}")
                           for j in range(4)]
                    for ktp in range(NTT // 2):
                        ps = mm_pool.tile([P, 1024], f32, tag="mm")
                        for s in range(2):
                            kt = ktp * 2 + s
                            nc.tensor.matmul(
                                ps[:, s * 512:(s + 1) * 512],
                                lhsT=kTg[:, kt * P:(kt + 1) * P],
                                rhs=qTh[:, qc * 512:(qc + 1) * 512],
                                start=True, stop=True)
                        pt = pt_pool.tile([P, 1024], bf16, tag="pt")
                        if masked:
                            for s in range(2):
                                kt = ktp * 2 + s
                                nc.scalar.activation(
                                    pt[:, s * 512:(s + 1) * 512],
                                    ps[:, s * 512:(s + 1) * 512],
                                    AF.Exp, bias=maskcol_t[:, kt:kt + 1])
                        else:
                            nc.scalar.activation(pt[:], ps[:], AF.Exp)
                        for s in range(2):
                            kt = ktp * 2 + s
                            for j in range(4):
                                nc.tensor.matmul(
                                    pvs[j][:, 0:P + 1],
                                    lhsT=pt[:, s * 512 + j * P:s * 512 + (j + 1) * P],
                                    rhs=v_aug[kt][:, g * (P + 1):(g + 1) * (P + 1)],
                                    start=(kt == 0), stop=(kt == NTT - 1))
                    for j in range(4):
                        qt = qc * 4 + j
                        rc = rc_pool.tile([P, 1], f32, tag="rc")
                        nc.vector.reciprocal(rc[:], pvs[j][:, P:P + 1])
                        nc.vector.tensor_scalar_mul(
                            attn_t[qt][:, h * P:(h + 1) * P],
                            pvs[j][:, 0:P], rc[:])

                # transpose the 4 finished t-tiles to d-major
                for j in range(4):
                    qt = qc * 4 + j
                    for dtile in range(NDT):
                        tps = pv_pool.tile([P, P], bf16, tag="pv")
                        nc.tensor.transpose(
                            tps[:], attn_t[qt][:, dtile * P:(dtile + 1) * P],
                            identity[:])
                        nc.vector.tensor_copy(aT[dtile][:, qt * P:(qt + 1) * P], tps[:])
                # o_proj for the 4 finished t-tiles
                for j in range(4):
                    tt = qc * 4 + j
                    for nchn in range(NCH):
                        ps = mm_pool.tile([P, 512], f32, tag="mm")
                        for dtile in range(NDT):
                            nc.tensor.matmul(
                                ps[:],
                                lhsT=aT[dtile][:, tt * P:(tt + 1) * P],
                                rhs=wo_t[dtile][:, nchn * 512:(nchn + 1) * 512],
                                start=(dtile == 0), stop=(dtile == NDT - 1))
                        osb = osb_pool.tile([P, 512], f32, tag="osb")
                        nc.vector.tensor_copy(osb[:], ps[:])
                        nc.sync.dma_start(
                            out=out[tt * P:(tt + 1) * P, nchn * 512:(nchn + 1) * 512],
                            in_=osb[:])

    nc.compile()
    return nc


def make_tables():
    inv_freq = 1.0 / (THETA ** (np.arange(0, HEAD_DIM, 2, dtype=np.float32)
                                / HEAD_DIM))          # [64]
    ang = np.arange(T, dtype=np.float32)[:, None] * inv_freq[None, :]  # [T, 64]
    cos = np.cos(ang).T.astype(np.float32)            # [64, T]
    sin = np.sin(ang).T.astype(np.float32)
    cos2 = np.concatenate([cos, cos], axis=0)         # [128, T]
    sinA = np.concatenate([-sin, sin], axis=0)        # [128, T]
    return (np.ascontiguousarray(cos2 * SCALE), np.ascontiguousarray(sinA * SCALE),
            np.ascontiguousarray(cos2), np.ascontiguousarray(sinA))


def make_in_maps(x, W_qkv, W_o, padding_mask, masked):
    cosq_v, sinq_v, cosk_v, sink_v = make_tables()
    in_maps = []
    for c in range(N_CORES):
        b, half = c // 2, c % 2
        q0 = half * QDIM
        k0 = NUM_HEADS * HEAD_DIM + half * GDIM
        v0 = NUM_HEADS * HEAD_DIM + QUERY_GROUPS * HEAD_DIM + half * GDIM
        wqk_v = np.concatenate(
            [W_qkv[:, q0:q0 + QDIM], W_qkv[:, k0:k0 + GDIM]], axis=1)
        m = {
            "xt": np.ascontiguousarray(x[b].T).astype(BF16),
            "wqk": np.ascontiguousarray(wqk_v).astype(BF16),
            "wv": np.ascontiguousarray(W_qkv[:, v0:v0 + GDIM]).astype(BF16),
            "wo": np.ascontiguousarray(W_o[half * QDIM:(half + 1) * QDIM, :]).astype(BF16),
            "cosq": cosq_v, "sinq": sinq_v, "cosk": cosk_v, "sink": sink_v,
        }
        if masked:
            bias = np.where(padding_mask[b], 0.0, -1e30).astype(np.float32)  # [T]
            m["maskcol"] = np.ascontiguousarray(
                bias.reshape(NTT, P).T).astype(np.float32)
        in_maps.append(m)
    return in_maps


_nc_cache = {}


def kernel(x, W_qkv, W_o, padding_mask, trace=False):
    from concourse.bass_utils import run_bass_kernel_spmd

    x = np.asarray(x)
    W_qkv = np.asarray(W_qkv)
    W_o = np.asarray(W_o)
    padding_mask = np.asarray(padding_mask)
    masked = not bool(padding_mask.all())

    if masked not in _nc_cache:
        _nc_cache[masked] = build_nc(masked)
    nc = _nc_cache[masked]

    in_maps = make_in_maps(x, W_qkv, W_o, padding_mask, masked)
    res = run_bass_kernel_spmd(
        nc, in_maps, core_ids=list(range(N_CORES)),
        trace=trace, trace_cores=[0] if trace else None)

    out = np.empty((B, T, D_MODEL), np.float32)
    for b in range(B):
        out[b] = res.results[2 * b]["out"] + res.results[2 * b + 1]["out"]
    kernel.last_exec_time_ns = res.exec_time_ns
    kernel.last_results = res
    return out



# revision 8
# speedup vs baseline: 1.0975x; 1.0975x over previous
"""Trainium2 Bass kernel for GQA attention (B=4, T=2048, D=2048, 16 heads / 4 kv groups, RoPE).

Sharding: 8 cores = 4 batches x 2 head-halves. Core c handles batch c//2 and
heads (c%2)*8..+8 with kv groups (c%2)*2..+2.

v2 structure (vs v1 baseline):
  - phase 1: per chunk [V-proj pairs -> K-proj -> Q-proj] channel-major with
    RoPE fused into psum eviction (DVE); V eviction moved to ACT; 1/sqrt(d)
    folded into the exp activation scale so only 2 unscaled rope tables load.
  - chunk 3 defers its Q-projection: the 8 Q m-groups are interleaved as PE
    filler into chunk-0's attention head slots (which are otherwise exp/ACT
    bound), after K3/V3 complete.
  - attention: per (qc,h): S^T tiles [k=128, q=512x2] -> exp(scale*s) on ACT
    -> PV via ones-augmented v (denominator in psum col 128), software
    pipelined one ktp ahead, with o_proj matmuls + PE transposes of the
    previous chunk interleaved as per-ktp filler so the PE never waits on ACT.
  - psum: "mm" 2x[128,1024] scores, "op" 2x[128,512] proj+oproj,
    "pv" 1x[128,1024] packed pvs regions at col {0,132,264,512} (+bf16
    transpose scratch bitcast at f32 cols 644/708) = exactly 8 banks.
  - DMA spread across sync/scalar/gpsimd queues; xc double-chunk buffered.
All matmuls bf16 with fp32 PSUM accumulation; output fp32, host sums halves.
"""

import numpy as np
import ml_dtypes

BF16 = ml_dtypes.bfloat16

D_MODEL = 2048
NUM_HEADS = 16
QUERY_GROUPS = 4
HEAD_DIM = 128
B = 4
T = 2048
THETA = 10000.0
SCALE = 0.08838834764831845
N_CORES = 8

P = 128
NH = NUM_HEADS // 2          # 8 q heads per core
NG = QUERY_GROUPS // 2       # 2 kv groups per core
QDIM = NH * HEAD_DIM         # 1024
GDIM = NG * HEAD_DIM         # 256
NKT = D_MODEL // P           # 16 contraction tiles over d_model
NTT = T // P                 # 16 tiles over sequence
NCH = T // 512               # 4 chunks of 512 over sequence
NDT = QDIM // P              # 8 head/dim tiles per core

# f32 col offsets of the 4 pv regions in the pv psum tile. j0/j1 share bank A
# (cols 0-511), j2/j3 share bank B (cols 512-1023); each bank's two regions
# form ONE psum accumulation group (start only on the bank's first matmul)
# because start=True clears the whole bank's has_written bits.
OFFJ = [0, 132, 512, 644]
SCRW = 64                    # f32 cols per bf16 [128,128] transpose scratch
VSTR = 2 * (P + 1)           # 258 bf16 cols per t-tile block in vbig


def build_nc(masked: bool):
    import concourse.bacc as bacc
    import concourse.tile as tile
    import concourse.mybir as mybir
    from concourse.masks import make_identity
    from contextlib import ExitStack

    dt = mybir.dt
    f32 = dt.float32
    bf16 = dt.bfloat16
    AF = mybir.ActivationFunctionType

    nc = bacc.Bacc("TRN2", target_bir_lowering=False, debug=False, num_devices=N_CORES)

    xt = nc.dram_tensor("xt", [D_MODEL, T], bf16, kind="ExternalInput")
    wq = nc.dram_tensor("wq", [D_MODEL, QDIM], bf16, kind="ExternalInput")
    wk = nc.dram_tensor("wk", [D_MODEL, GDIM], bf16, kind="ExternalInput")
    wv = nc.dram_tensor("wv", [D_MODEL, GDIM], bf16, kind="ExternalInput")
    wo = nc.dram_tensor("wo", [QDIM, D_MODEL], bf16, kind="ExternalInput")
    cos2 = nc.dram_tensor("cos2", [P, T], f32, kind="ExternalInput")
    sinA = nc.dram_tensor("sinA", [P, T], f32, kind="ExternalInput")
    if masked:
        maskcol = nc.dram_tensor("maskcol", [P, NTT], f32, kind="ExternalInput")
    out = nc.dram_tensor("out", [T, D_MODEL], f32, kind="ExternalOutput")

    with tile.TileContext(nc) as tc:
        with ExitStack() as ctx:
            psum = ctx.enter_context(tc.tile_pool(name="ps", bufs=1, space="PSUM"))
            constp = ctx.enter_context(tc.tile_pool(name="const", bufs=1))
            qkT_pool = ctx.enter_context(tc.tile_pool(name="qkT", bufs=NH + NG))
            vbig_pool = ctx.enter_context(tc.tile_pool(name="vbig", bufs=1))
            attn_pool = ctx.enter_context(tc.tile_pool(name="attn", bufs=NTT))
            pt_pool = ctx.enter_context(tc.tile_pool(name="pt", bufs=4))
            rc_pool = ctx.enter_context(tc.tile_pool(name="rc", bufs=8))

            def mm_tile():
                return psum.tile([P, 1024], f32, tag="mm", bufs=2, name="mmt")

            def op_tile():
                return psum.tile([P, 512], f32, tag="op", bufs=2, name="opt")

            def pv_tile():
                return psum.tile([P, 1024], f32, tag="pv", bufs=1, name="pvt")

            identity = constp.tile([P, P], bf16, tag="identity")
            make_identity(nc, identity[:])
            dummy = constp.tile([P, 256], bf16, tag="dummy")
            nc.vector.memset(dummy[:], 0.0)
            if masked:
                maskcol_t = constp.tile([P, NTT], f32, tag="maskcol")
                nc.scalar.dma_start(out=maskcol_t[:], in_=maskcol[:, :])

            # persistent bf16 tensors
            # qkT[0..7] = q heads, qkT[8..9] = k groups; [d=128, T] channel-major
            qkT = [qkT_pool.tile([P, T], bf16, tag="qkT", name=f"qkT{i}")
                   for i in range(NH + NG)]
            # vbig: per t-tile block of 258 cols: [v_g0(128) | ones | v_g1(128) | ones]
            vbig = vbig_pool.tile([P, NTT * VSTR], bf16, tag="vbig")
            for i in range(NTT):
                for g in range(NG):
                    c = i * VSTR + g * (P + 1) + P
                    nc.vector.memset(vbig[:, c:c + 1], 1.0)
            attn_t = [attn_pool.tile([P, QDIM], bf16, tag="attn", name=f"attn{i}")
                      for i in range(NTT)]

            # ---------------- warmup (HAM + cover initial DMA latency) ------
            warm = op_tile()
            for i in range(48):
                nc.tensor.matmul(warm[:, 0:256], lhsT=identity[:], rhs=dummy[:],
                                 start=(i == 0), stop=(i == 47))
            wsink = constp.tile([P, 16], f32, tag="wsink")
            nc.vector.tensor_copy(wsink[:], warm[:, 0:16])

            # ---------------- shared attention machinery -------------------
            # slot state for the attention pipeline
            st = {"pvs": None, "pvs_hq": None, "cur_pvs": None}

            def normalize(qc, h, pvs_prev):
                # attn_t[qt][:, h*128:+128] = pvs[:, j] / denom  (DVE)
                for j in range(4):
                    qt = qc * 4 + j
                    rc = rc_pool.tile([P, 1], f32, tag="rc", name="rc")
                    nc.vector.reciprocal(
                        rc[:], pvs_prev[:, OFFJ[j] + P:OFFJ[j] + P + 1])
                    nc.vector.tensor_scalar_mul(
                        attn_t[qt][:, h * P:(h + 1) * P],
                        pvs_prev[:, OFFJ[j]:OFFJ[j] + P], rc[:])

            def attn_slot(qc, h, fillers):
                """One head slot: scores+exp+PV pipelined 1 ktp deep, with
                `fillers` (list of <=8 callables of ~0.5-1us PE work each)
                emitted at the 8 interleave points."""
                g = h // 4
                kT = qkT[NH + g]
                qT = qkT[h]
                c0 = qc * 512

                # normalize the previous head's pvs (frees the pv psum slot)
                if st["pvs"] is not None:
                    pq, ph = st["pvs_hq"]
                    normalize(pq, ph, st["pvs"])
                    st["pvs"] = None

                pvs = pv_tile()
                st["cur_pvs"] = pvs

                ps_l = [None] * 8
                pt_l = [None] * 8

                def emit_mms(ktp):
                    ps = mm_tile()
                    ps_l[ktp] = ps
                    for s in range(2):
                        kt = ktp * 2 + s
                        nc.tensor.matmul(
                            ps[:, s * 512:(s + 1) * 512],
                            lhsT=kT[:, kt * P:(kt + 1) * P],
                            rhs=qT[:, c0:c0 + 512],
                            start=True, stop=True)
                    pt = pt_pool.tile([P, 1024], bf16, tag="pt", name="pt")
                    pt_l[ktp] = pt
                    if masked:
                        for s in range(2):
                            kt = ktp * 2 + s
                            nc.scalar.activation(
                                pt[:, s * 512:(s + 1) * 512],
                                ps[:, s * 512:(s + 1) * 512],
                                AF.Exp, bias=maskcol_t[:, kt:kt + 1], scale=SCALE)
                    else:
                        nc.scalar.activation(pt[:], ps[:], AF.Exp, scale=SCALE)

                def emit_pv(ktp):
                    pt = pt_l[ktp]
                    for s in range(2):
                        kt = ktp * 2 + s
                        for j in range(4):
                            # j0/j1 (bank A) and j2/j3 (bank B) each form one
                            # accumulation group: start only on the bank's
                            # first matmul, stop on its last.
                            nc.tensor.matmul(
                                pvs[:, OFFJ[j]:OFFJ[j] + P + 1],
                                lhsT=pt[:, s * 512 + j * P:s * 512 + (j + 1) * P],
                                rhs=vbig[:, kt * VSTR + g * (P + 1):
                                         kt * VSTR + (g + 1) * (P + 1)],
                                start=(kt == 0 and j % 2 == 0),
                                stop=(kt == NTT - 1 and j % 2 == 1),
                                skip_group_check=True)

                emit_mms(0)
                for ktp in range(8):
                    if ktp < 7:
                        emit_mms(ktp + 1)
                    if ktp < len(fillers) and fillers[ktp] is not None:
                        fillers[ktp]()
                    emit_pv(ktp)

                st["pvs"] = pvs
                st["pvs_hq"] = (qc, h)

            # ---------------- phase 1: projections + rope -------------------
            with ExitStack() as ph1:
                wq_pool = ph1.enter_context(tc.tile_pool(name="wq", bufs=NKT))
                wk_pool = ph1.enter_context(tc.tile_pool(name="wk", bufs=NKT))
                wv_pool = ph1.enter_context(tc.tile_pool(name="wv", bufs=NKT))
                xc_pool = ph1.enter_context(tc.tile_pool(name="xc", bufs=32))
                tab_pool = ph1.enter_context(tc.tile_pool(name="tab", bufs=1))
                tmp_pool = ph1.enter_context(tc.tile_pool(name="rtmp", bufs=2))

                # --- DMA emission, spread across queues ---
                # scalar queue: wv, wk, 4 xc0 tiles, tables
                wv_t = []
                for kt in range(NKT):
                    tl = wv_pool.tile([P, GDIM], bf16, tag="wv", name=f"wvt{kt}")
                    nc.scalar.dma_start(out=tl[:], in_=wv[kt * P:(kt + 1) * P, :])
                    wv_t.append(tl)
                wk_t = []
                for kt in range(NKT):
                    tl = wk_pool.tile([P, GDIM], bf16, tag="wk", name=f"wkt{kt}")
                    nc.scalar.dma_start(out=tl[:], in_=wk[kt * P:(kt + 1) * P, :])
                    wk_t.append(tl)

                xc_tiles = {}

                def load_xc(nch, kts, eng):
                    c0 = nch * 512
                    for kt in kts:
                        tl = xc_pool.tile([P, 512], bf16, tag="xc", name=f"xc{nch}_{kt}")
                        eng.dma_start(out=tl[:], in_=xt[kt * P:(kt + 1) * P, c0:c0 + 512])
                        xc_tiles[(nch, kt)] = tl

                # chunk-0 x: 12 tiles on sync, 4 on scalar
                load_xc(0, range(12), nc.sync)
                load_xc(0, range(12, 16), nc.scalar)
                cos_t = tab_pool.tile([P, T], f32, tag="cos", name="cos_t")
                nc.scalar.dma_start(out=cos_t[:], in_=cos2[:, :])
                sin_t = tab_pool.tile([P, T], f32, tag="sin", name="sin_t")
                nc.scalar.dma_start(out=sin_t[:], in_=sinA[:, :])
                # sync queue: wq then later chunks' xc
                wq_t = []
                for kt in range(NKT):
                    tl = wq_pool.tile([P, QDIM], bf16, tag="wq", name=f"wqt{kt}")
                    nc.sync.dma_start(out=tl[:], in_=wq[kt * P:(kt + 1) * P, :])
                    wq_t.append(tl)
                load_xc(1, range(NKT), nc.sync)

                def vproj_pair(c, pair):
                    # two t-tiles of 128 into one [128,512] psum; ACT evicts
                    ps = op_tile()
                    for tl_i in (0, 1):
                        for kt in range(NKT):
                            nc.tensor.matmul(
                                ps[:, tl_i * 256:tl_i * 256 + GDIM],
                                lhsT=xc_tiles[(c, kt)][:, (pair * 2 + tl_i) * P:
                                                       (pair * 2 + tl_i + 1) * P],
                                rhs=wv_t[kt][:],
                                start=(kt == 0), stop=(kt == NKT - 1))
                    for tl_i in (0, 1):
                        tglob = c * 4 + pair * 2 + tl_i
                        for g in range(NG):
                            nc.scalar.activation(
                                vbig[:, tglob * VSTR + g * (P + 1):
                                     tglob * VSTR + g * (P + 1) + P],
                                ps[:, tl_i * 256 + g * P:tl_i * 256 + (g + 1) * P],
                                AF.Copy)

                def rope_evict(m, ps, c0):
                    # qkT[m][:, c0:c0+512] = ps*cos2 + rot(ps)*sinA   (DVE, f32)
                    t1 = tmp_pool.tile([P, 512], f32, tag="t1", name="t1")
                    t2 = tmp_pool.tile([P, 512], f32, tag="t2", name="t2")
                    h2 = P // 2
                    nc.vector.tensor_mul(t1[:], ps[:], cos_t[:, c0:c0 + 512])
                    nc.vector.tensor_mul(
                        t2[0:h2, :], ps[h2:P, :], sin_t[0:h2, c0:c0 + 512])
                    nc.vector.tensor_mul(
                        t2[h2:P, :], ps[0:h2, :], sin_t[h2:P, c0:c0 + 512])
                    nc.vector.tensor_add(qkT[m][:, c0:c0 + 512], t1[:], t2[:])

                def kq_mgroup(c, m):
                    # m 0..7 -> q head m (wq cols), m 8..9 -> k group (wk cols)
                    ps = op_tile()
                    for kt in range(NKT):
                        if m < NH:
                            lhsT = wq_t[kt][:, m * P:(m + 1) * P]
                        else:
                            lhsT = wk_t[kt][:, (m - NH) * P:(m - NH + 1) * P]
                        nc.tensor.matmul(
                            ps[:], lhsT=lhsT,
                            rhs=xc_tiles[(c, kt)][:],
                            start=(kt == 0), stop=(kt == NKT - 1))
                    rope_evict(m, ps, c * 512)

                for c in range(NCH):
                    if 1 <= c < NCH - 1:
                        load_xc(c + 1, range(NKT), nc.sync)
                    vproj_pair(c, 0)
                    vproj_pair(c, 1)
                    # K first (enables attention right after chunk 3's K)
                    for m in (NH, NH + 1):
                        kq_mgroup(c, m)
                    if c < NCH - 1:
                        for m in range(NH):
                            kq_mgroup(c, m)

                # ---- hybrid: chunk-0 attention, Q3 m-groups as PE filler ----
                q3 = {}

                def q3_unit(h, quarter):
                    # quarter of Q-projection m-group h for chunk 3 (4 MMs)
                    c = NCH - 1
                    if quarter == 0:
                        q3[h] = op_tile()
                    ps = q3[h]
                    for kt in range(quarter * 4, quarter * 4 + 4):
                        nc.tensor.matmul(
                            ps[:], lhsT=wq_t[kt][:, h * P:(h + 1) * P],
                            rhs=xc_tiles[(c, kt)][:],
                            start=(kt == 0), stop=(kt == NKT - 1))
                    if quarter == 3:
                        rope_evict(h, ps, c * 512)

                for h in range(NH):
                    fillers = [None] * 8
                    for q in range(4):
                        fillers[1 + 2 * q] = (lambda hh=h, qq=q: q3_unit(hh, qq))
                    attn_slot(0, h, fillers)

            # ---------------- phase 2: chunks 1-3 + o_proj ------------------
            wo_pool = ctx.enter_context(tc.tile_pool(name="wo", bufs=NDT))
            aT_pool = ctx.enter_context(tc.tile_pool(name="aT", bufs=1))
            osb_pool = ctx.enter_context(tc.tile_pool(name="osb", bufs=6))

            # aTbig[:, dtile*T + qt*128 : +128] = attn_t[qt][:, dtile].T
            aTbig = aT_pool.tile([P, NDT * T], bf16, tag="aT")
            wo_t = []
            for dtile in range(NDT):
                tl = wo_pool.tile([P, D_MODEL], bf16, tag="wo", name=f"wot{dtile}")
                wo_t.append(tl)
            for dtile in range(NDT):
                nc.gpsimd.dma_start(out=wo_t[dtile][:, 0:1024],
                                    in_=wo[dtile * P:(dtile + 1) * P, 0:1024])
            for dtile in range(NDT):
                nc.gpsimd.dma_start(out=wo_t[dtile][:, 1024:2048],
                                    in_=wo[dtile * P:(dtile + 1) * P, 1024:2048])

            def transp_unit(qcp, tt, pair):
                # transpose attn_t[qt] dtiles (2*pair, 2*pair+1) into aTbig,
                # via bf16 scratch bitcast-carved from a fresh op psum tile
                # (an op tile owns a full bank, so the transpose's start=True
                # bank-clear cannot disturb any in-flight accumulation).
                qt = qcp * 4 + tt
                scrt = op_tile()
                for s2 in range(2):
                    dtile = pair * 2 + s2
                    scr = scrt[:, s2 * SCRW:(s2 + 1) * SCRW].bitcast(bf16)
                    nc.tensor.transpose(
                        scr, attn_t[qt][:, dtile * P:(dtile + 1) * P], identity[:])
                    nc.vector.tensor_copy(
                        aTbig[:, dtile * T + qt * P:dtile * T + (qt + 1) * P], scr)

            op_state = {}

            def oproj_unit(tt, nchn, half):
                # half 0: dtiles 0-3 (start); half 1: dtiles 4-7 (stop+evict)
                if half == 0:
                    op_state[(tt, nchn)] = op_tile()
                ps = op_state[(tt, nchn)]
                for dtile in range(half * 4, half * 4 + 4):
                    nc.tensor.matmul(
                        ps[:],
                        lhsT=aTbig[:, dtile * T + tt * P:dtile * T + (tt + 1) * P],
                        rhs=wo_t[dtile][:, nchn * 512:(nchn + 1) * 512],
                        start=(dtile == 0), stop=(dtile == NDT - 1))
                if half == 1:
                    del op_state[(tt, nchn)]
                    osb = osb_pool.tile([P, 512], f32, tag="osb", name="osb")
                    nc.vector.tensor_copy(osb[:], ps[:])
                    nc.gpsimd.dma_start(
                        out=out[tt * P:(tt + 1) * P, nchn * 512:(nchn + 1) * 512],
                        in_=osb[:])

            # filler scheduling: per chunk qc (1..3), slots h=0..7 carry
            # transposes of chunk qc-1 (slot h<4 -> tt=h, 4 pair-units at
            # points 4-7) and o_proj groups of chunk qc-1 (2 units each) from
            # a readiness queue.
            ready_groups = []   # (tt_glob, nchn) ready once tt transposed

            for qc in range(1, NCH):
                qcp = qc - 1
                for h in range(NH):
                    fillers = []
                    trans = []
                    if h < 4:
                        trans = [(lambda t=h, p=p2: transp_unit(qcp, t, p))
                                 for p2 in range(4)]
                    n_op = min(8 - len(trans), 4)
                    opu = []
                    while ready_groups and len(opu) + 2 <= n_op:
                        ttg, nchn = ready_groups.pop(0)
                        opu.append(lambda a=ttg, b=nchn: oproj_unit(a, b, 0))
                        opu.append(lambda a=ttg, b=nchn: oproj_unit(a, b, 1))
                    # op units first (points 0..), transposes at the tail
                    fillers = opu + trans
                    attn_slot(qc, h, fillers)
                    if h < 4:
                        ttg = qcp * 4 + h
                        for nchn in range(NCH):
                            ready_groups.append((ttg, nchn))

            # ---------------- tail: last normalize + transposes + o_proj ----
            pq, ph = st["pvs_hq"]
            pvs_last = st["pvs"]
            normalize(pq, ph, pvs_last)
            st["pvs"] = None
            qcp = NCH - 1
            for tt in range(4):
                for p2 in range(4):
                    transp_unit(qcp, tt, p2)
                for nchn in range(NCH):
                    ready_groups.append((qcp * 4 + tt, nchn))
            for ttg, nchn in ready_groups:
                oproj_unit(ttg, nchn, 0)
                oproj_unit(ttg, nchn, 1)

    nc.compile()
    return nc


def make_tables():
    inv_freq = 1.0 / (THETA ** (np.arange(0, HEAD_DIM, 2, dtype=np.float32)
                                / HEAD_DIM))          # [64]
    ang = np.arange(T, dtype=np.float32)[:, None] * inv_freq[None, :]  # [T, 64]
    cos = np.cos(ang).T.astype(np.float32)            # [64, T]
    sin = np.sin(ang).T.astype(np.float32)
    cos2 = np.concatenate([cos, cos], axis=0)         # [128, T]
    sinA = np.concatenate([-sin, sin], axis=0)        # [128, T]
    return np.ascontiguousarray(cos2), np.ascontiguousarray(sinA)


def make_in_maps(x, W_qkv, W_o, padding_mask, masked):
    cos2_v, sinA_v = make_tables()
    in_maps = []
    for c in range(N_CORES):
        b, half = c // 2, c % 2
        q0 = half * QDIM
        k0 = NUM_HEADS * HEAD_DIM + half * GDIM
        v0 = NUM_HEADS * HEAD_DIM + QUERY_GROUPS * HEAD_DIM + half * GDIM
        m = {
            "xt": np.ascontiguousarray(x[b].T).astype(BF16),
            "wq": np.ascontiguousarray(W_qkv[:, q0:q0 + QDIM]).astype(BF16),
            "wk": np.ascontiguousarray(W_qkv[:, k0:k0 + GDIM]).astype(BF16),
            "wv": np.ascontiguousarray(W_qkv[:, v0:v0 + GDIM]).astype(BF16),
            "wo": np.ascontiguousarray(W_o[half * QDIM:(half + 1) * QDIM, :]).astype(BF16),
            "cos2": cos2_v, "sinA": sinA_v,
        }
        if masked:
            bias = np.where(padding_mask[b], 0.0, -1e30).astype(np.float32)  # [T]
            m["maskcol"] = np.ascontiguousarray(
                bias.reshape(NTT, P).T).astype(np.float32)
        in_maps.append(m)
    return in_maps


_nc_cache = {}


def kernel(x, W_qkv, W_o, padding_mask, trace=False):
    from concourse.bass_utils import run_bass_kernel_spmd

    x = np.asarray(x)
    W_qkv = np.asarray(W_qkv)
    W_o = np.asarray(W_o)
    padding_mask = np.asarray(padding_mask)
    masked = not bool(padding_mask.all())

    if masked not in _nc_cache:
        _nc_cache[masked] = build_nc(masked)
    nc = _nc_cache[masked]

    in_maps = make_in_maps(x, W_qkv, W_o, padding_mask, masked)
    res = run_bass_kernel_spmd(
        nc, in_maps, core_ids=list(range(N_CORES)),
        trace=trace, trace_cores=[0] if trace else None)

    out = np.empty((B, T, D_MODEL), np.float32)
    for b in range(B):
        out[b] = res.results[2 * b]["out"] + res.results[2 * b + 1]["out"]
    kernel.last_exec_time_ns = res.exec_time_ns
    kernel.last_results = res
    return out


# revision 14
# speedup vs baseline: 1.1271x; 1.0269x over previous
"""Trainium2 Bass kernel for GQA attention (B=4, T=2048, D=2048, 16 heads / 4 kv groups, RoPE).

Sharding: 8 cores = 4 batches x 2 head-halves. Core c handles batch c//2 and
heads (c%2)*8..+8 with kv groups (c%2)*2..+2.

v2 structure (vs v1 baseline):
  - phase 1: per chunk [V-proj pairs -> K-proj -> Q-proj] channel-major with
    RoPE fused into psum eviction (DVE); V eviction moved to ACT; 1/sqrt(d)
    folded into the exp activation scale so only 2 unscaled rope tables load.
  - chunk 3 defers its Q-projection: the 8 Q m-groups are interleaved as PE
    filler into chunk-0's attention head slots (which are otherwise exp/ACT
    bound), after K3/V3 complete.
  - attention: per (qc,h): S^T tiles [k=128, q=512x2] -> exp(scale*s) on ACT
    -> PV via ones-augmented v (denominator in psum col 128), software
    pipelined one ktp ahead, with o_proj matmuls + PE transposes of the
    previous chunk interleaved as per-ktp filler so the PE never waits on ACT.
  - psum: "mm" 2x[128,1024] scores, "op" 2x[128,512] proj+oproj,
    "pv" 1x[128,1024] packed pvs regions at col {0,132,264,512} (+bf16
    transpose scratch bitcast at f32 cols 644/708) = exactly 8 banks.
  - DMA spread across sync/scalar/gpsimd queues; xc double-chunk buffered.
All matmuls bf16 with fp32 PSUM accumulation; output fp32, host sums halves.
"""

import numpy as np
import ml_dtypes

BF16 = ml_dtypes.bfloat16

D_MODEL = 2048
NUM_HEADS = 16
QUERY_GROUPS = 4
HEAD_DIM = 128
B = 4
T = 2048
THETA = 10000.0
SCALE = 0.08838834764831845
N_CORES = 8

P = 128
NH = NUM_HEADS // 2          # 8 q heads per core
NG = QUERY_GROUPS // 2       # 2 kv groups per core
QDIM = NH * HEAD_DIM         # 1024
GDIM = NG * HEAD_DIM         # 256
NKT = D_MODEL // P           # 16 contraction tiles over d_model
NTT = T // P                 # 16 tiles over sequence
NCH = T // 512               # 4 chunks of 512 over sequence
NDT = QDIM // P              # 8 head/dim tiles per core

# f32 col offsets of the 4 pv regions in the pv psum tile. j0/j1 share bank A
# (cols 0-511), j2/j3 share bank B (cols 512-1023); each bank's two regions
# form ONE psum accumulation group (start only on the bank's first matmul)
# because start=True clears the whole bank's has_written bits.
OFFJ = [0, 132, 512, 644]
SCRW = 64                    # f32 cols per bf16 [128,128] transpose scratch
VSTR = 2 * (P + 1)           # 258 bf16 cols per t-tile block in vbig


def build_nc(masked: bool):
    import concourse.bacc as bacc
    import concourse.tile as tile
    import concourse.mybir as mybir
    from concourse.masks import make_identity
    from contextlib import ExitStack

    dt = mybir.dt
    f32 = dt.float32
    bf16 = dt.bfloat16
    AF = mybir.ActivationFunctionType

    nc = bacc.Bacc("TRN2", target_bir_lowering=False, debug=False, num_devices=N_CORES)

    xt = nc.dram_tensor("xt", [D_MODEL, T], bf16, kind="ExternalInput")
    wq = nc.dram_tensor("wq", [D_MODEL, QDIM], bf16, kind="ExternalInput")
    # wv|wk packed so the tiles have 1KB lines (512B-line DMAs run ~2x slower)
    wvk = nc.dram_tensor("wvk", [D_MODEL, 2 * GDIM], bf16, kind="ExternalInput")
    wo = nc.dram_tensor("wo", [QDIM, D_MODEL], bf16, kind="ExternalInput")
    cos2 = nc.dram_tensor("cos2", [P, T], f32, kind="ExternalInput")
    sinA = nc.dram_tensor("sinA", [P, T], f32, kind="ExternalInput")
    if masked:
        maskcol = nc.dram_tensor("maskcol", [P, NTT], f32, kind="ExternalInput")
    out = nc.dram_tensor("out", [T, D_MODEL], f32, kind="ExternalOutput")

    with tile.TileContext(nc) as tc:
        with ExitStack() as ctx:
            psum = ctx.enter_context(tc.tile_pool(name="ps", bufs=1, space="PSUM"))
            constp = ctx.enter_context(tc.tile_pool(name="const", bufs=1))
            qkT_pool = ctx.enter_context(tc.tile_pool(name="qkT", bufs=NH + NG))
            vbig_pool = ctx.enter_context(tc.tile_pool(name="vbig", bufs=1))
            attn_pool = ctx.enter_context(tc.tile_pool(name="attn", bufs=NTT))
            pt_pool = ctx.enter_context(tc.tile_pool(name="pt", bufs=4))
            rc_pool = ctx.enter_context(tc.tile_pool(name="rc", bufs=8))

            def mm_tile():
                return psum.tile([P, 1024], f32, tag="mm", bufs=2, name="mmt")

            def op_tile():
                return psum.tile([P, 512], f32, tag="op", bufs=2, name="opt")

            def pv_tile():
                return psum.tile([P, 1024], f32, tag="pv", bufs=1, name="pvt")

            identity = constp.tile([P, P], bf16, tag="identity")
            make_identity(nc, identity[:])
            dummy = constp.tile([P, 256], bf16, tag="dummy")
            nc.vector.memset(dummy[:], 0.0)
            if masked:
                maskcol_t = constp.tile([P, NTT], f32, tag="maskcol")
                nc.gpsimd.dma_start(out=maskcol_t[:], in_=maskcol[:, :])

            # persistent bf16 tensors
            # qkT[0..7] = q heads, qkT[8..9] = k groups; [d=128, T] channel-major
            qkT = [qkT_pool.tile([P, T], bf16, tag="qkT", name=f"qkT{i}")
                   for i in range(NH + NG)]
            # vbig: per t-tile block of 258 cols: [v_g0(128) | ones | v_g1(128) | ones]
            vbig = vbig_pool.tile([P, NTT * VSTR], bf16, tag="vbig")
            for i in range(NTT):
                for g in range(NG):
                    c = i * VSTR + g * (P + 1) + P
                    nc.vector.memset(vbig[:, c:c + 1], 1.0)
            attn_t = [attn_pool.tile([P, QDIM], bf16, tag="attn", name=f"attn{i}")
                      for i in range(NTT)]

            # ---------------- warmup (HAM + cover initial DMA latency) ------
            warm = op_tile()
            for i in range(48):
                nc.tensor.matmul(warm[:, 0:256], lhsT=identity[:], rhs=dummy[:],
                                 start=(i == 0), stop=(i == 47))
            wsink = constp.tile([P, 16], f32, tag="wsink")
            nc.vector.tensor_copy(wsink[:], warm[:, 0:16])

            # ---------------- shared attention machinery -------------------
            # slot state for the attention pipeline
            st = {"pvs": None, "pvs_hq": None, "cur_pvs": None}

            def normalize(qc, h, pvs_prev):
                # attn_t[qt][:, h*128:+128] = pvs[:, j] / denom  (DVE)
                for j in range(4):
                    qt = qc * 4 + j
                    rc = rc_pool.tile([P, 1], f32, tag="rc", name="rc")
                    nc.vector.reciprocal(
                        rc[:], pvs_prev[:, OFFJ[j] + P:OFFJ[j] + P + 1])
                    nc.vector.tensor_scalar_mul(
                        attn_t[qt][:, h * P:(h + 1) * P],
                        pvs_prev[:, OFFJ[j]:OFFJ[j] + P], rc[:])

            def attn_slot(qc, h, fillers):
                """One head slot: scores+exp+PV pipelined 1 ktp deep, with
                `fillers` (list of <=8 callables of ~0.5-1us PE work each)
                emitted at the 8 interleave points."""
                g = h // 4
                kT = qkT[NH + g]
                qT = qkT[h]
                c0 = qc * 512

                # normalize the previous head's pvs (frees the pv psum slot)
                if st["pvs"] is not None:
                    pq, ph = st["pvs_hq"]
                    normalize(pq, ph, st["pvs"])
                    st["pvs"] = None

                pvs = pv_tile()
                st["cur_pvs"] = pvs

                ps_l = [None] * 8
                pt_l = [None] * 8

                def emit_mms(ktp):
                    ps = mm_tile()
                    ps_l[ktp] = ps
                    for s in range(2):
                        kt = ktp * 2 + s
                        nc.tensor.matmul(
                            ps[:, s * 512:(s + 1) * 512],
                            lhsT=kT[:, kt * P:(kt + 1) * P],
                            rhs=qT[:, c0:c0 + 512],
                            start=True, stop=True)
                    pt = pt_pool.tile([P, 1024], bf16, tag="pt", name="pt")
                    pt_l[ktp] = pt
                    if masked:
                        for s in range(2):
                            kt = ktp * 2 + s
                            nc.scalar.activation(
                                pt[:, s * 512:(s + 1) * 512],
                                ps[:, s * 512:(s + 1) * 512],
                                AF.Exp, bias=maskcol_t[:, kt:kt + 1], scale=SCALE)
                    else:
                        nc.scalar.activation(pt[:], ps[:], AF.Exp, scale=SCALE)

                def emit_pv(ktp):
                    pt = pt_l[ktp]
                    for s in range(2):
                        kt = ktp * 2 + s
                        for j in range(4):
                            # j0/j1 (bank A) and j2/j3 (bank B) each form one
                            # accumulation group: start only on the bank's
                            # first matmul, stop on its last.
                            nc.tensor.matmul(
                                pvs[:, OFFJ[j]:OFFJ[j] + P + 1],
                                lhsT=pt[:, s * 512 + j * P:s * 512 + (j + 1) * P],
                                rhs=vbig[:, kt * VSTR + g * (P + 1):
                                         kt * VSTR + (g + 1) * (P + 1)],
                                start=(kt == 0 and j % 2 == 0),
                                stop=(kt == NTT - 1 and j % 2 == 1),
                                skip_group_check=True)

                emit_mms(0)
                for ktp in range(8):
                    if ktp < 7:
                        emit_mms(ktp + 1)
                    if ktp < len(fillers) and fillers[ktp] is not None:
                        fillers[ktp]()
                    emit_pv(ktp)

                st["pvs"] = pvs
                st["pvs_hq"] = (qc, h)

            # ---------------- phase 1: projections + rope -------------------
            with ExitStack() as ph1:
                wq_pool = ph1.enter_context(tc.tile_pool(name="wq", bufs=NKT))
                wvk_pool = ph1.enter_context(tc.tile_pool(name="wvk", bufs=NKT))
                xc_pool = ph1.enter_context(tc.tile_pool(name="xc", bufs=32))
                tab_pool = ph1.enter_context(tc.tile_pool(name="tab", bufs=1))
                tmp_pool = ph1.enter_context(tc.tile_pool(name="rtmp", bufs=2))

                # --- DMA emission: sync + gpsimd queues only, so the scalar
                # (ACT) instruction stream stays free for psum evictions ---
                xc_tiles = {}

                def load_xc(nch, kts, eng):
                    c0 = nch * 512
                    for kt in kts:
                        tl = xc_pool.tile([P, 512], bf16, tag="xc", name=f"xc{nch}_{kt}")
                        eng.dma_start(out=tl[:], in_=xt[kt * P:(kt + 1) * P, c0:c0 + 512])
                        xc_tiles[(nch, kt)] = tl

                # gpsimd queue: wvk, tables, wq kt 8-15
                wvk_t = []
                for kt in range(NKT):
                    tl = wvk_pool.tile([P, 2 * GDIM], bf16, tag="wvk", name=f"wvkt{kt}")
                    nc.gpsimd.dma_start(out=tl[:], in_=wvk[kt * P:(kt + 1) * P, :])
                    wvk_t.append(tl)
                wv_t = [tl[:, 0:GDIM] for tl in wvk_t]
                wk_t = [tl[:, GDIM:2 * GDIM] for tl in wvk_t]
                cos_t = tab_pool.tile([P, T], f32, tag="cos", name="cos_t")
                nc.gpsimd.dma_start(out=cos_t[:], in_=cos2[:, :])
                sin_t = tab_pool.tile([P, T], f32, tag="sin", name="sin_t")
                nc.gpsimd.dma_start(out=sin_t[:], in_=sinA[:, :])
                # sync queue: xc0, wq kt 0-7, xc1
                load_xc(0, range(NKT), nc.sync)
                wq_t = [wq_pool.tile([P, QDIM], bf16, tag="wq", name=f"wqt{kt}")
                        for kt in range(NKT)]
                for kt in range(8):
                    nc.sync.dma_start(out=wq_t[kt][:], in_=wq[kt * P:(kt + 1) * P, :])
                for kt in range(8, NKT):
                    nc.gpsimd.dma_start(out=wq_t[kt][:], in_=wq[kt * P:(kt + 1) * P, :])
                load_xc(1, range(NKT), nc.sync)

                def vproj_pair(c, pair):
                    # two t-tiles of 128 into one [128,512] psum; ACT evicts
                    ps = op_tile()
                    for tl_i in (0, 1):
                        for kt in range(NKT):
                            nc.tensor.matmul(
                                ps[:, tl_i * 256:tl_i * 256 + GDIM],
                                lhsT=xc_tiles[(c, kt)][:, (pair * 2 + tl_i) * P:
                                                       (pair * 2 + tl_i + 1) * P],
                                rhs=wv_t[kt],
                                start=(kt == 0), stop=(kt == NKT - 1))
                    for tl_i in (0, 1):
                        tglob = c * 4 + pair * 2 + tl_i
                        for g in range(NG):
                            nc.scalar.activation(
                                vbig[:, tglob * VSTR + g * (P + 1):
                                     tglob * VSTR + g * (P + 1) + P],
                                ps[:, tl_i * 256 + g * P:tl_i * 256 + (g + 1) * P],
                                AF.Copy)

                def rope_evict(m, ps, c0):
                    # qkT[m][:, c0:c0+512] = ps*cos2 + rot(ps)*sinA   (DVE, f32)
                    t1 = tmp_pool.tile([P, 512], f32, tag="t1", name="t1")
                    t2 = tmp_pool.tile([P, 512], f32, tag="t2", name="t2")
                    h2 = P // 2
                    nc.vector.tensor_mul(t1[:], ps[:], cos_t[:, c0:c0 + 512])
                    nc.vector.tensor_mul(
                        t2[0:h2, :], ps[h2:P, :], sin_t[0:h2, c0:c0 + 512])
                    nc.vector.tensor_mul(
                        t2[h2:P, :], ps[0:h2, :], sin_t[h2:P, c0:c0 + 512])
                    nc.vector.tensor_add(qkT[m][:, c0:c0 + 512], t1[:], t2[:])

                def kq_mgroup(c, m):
                    # m 0..7 -> q head m (wq cols), m 8..9 -> k group (wk cols)
                    ps = op_tile()
                    for kt in range(NKT):
                        if m < NH:
                            lhsT = wq_t[kt][:, m * P:(m + 1) * P]
                        else:
                            lhsT = wk_t[kt][:, (m - NH) * P:(m - NH + 1) * P]
                        nc.tensor.matmul(
                            ps[:], lhsT=lhsT,
                            rhs=xc_tiles[(c, kt)][:],
                            start=(kt == 0), stop=(kt == NKT - 1))
                    rope_evict(m, ps, c * 512)

                for c in range(NCH):
                    if 1 <= c < NCH - 1:
                        load_xc(c + 1, range(NKT), nc.sync)
                    vproj_pair(c, 0)
                    vproj_pair(c, 1)
                    # K first (enables attention right after chunk 3's K)
                    for m in (NH, NH + 1):
                        kq_mgroup(c, m)
                    if c < NCH - 1:
                        for m in range(NH):
                            kq_mgroup(c, m)

                # ---- hybrid: chunk-0 attention, Q3 m-groups as PE filler ----
                q3 = {}

                def q3_unit(h, quarter):
                    # quarter of Q-projection m-group h for chunk 3 (4 MMs)
                    c = NCH - 1
                    if quarter == 0:
                        q3[h] = op_tile()
                    ps = q3[h]
                    for kt in range(quarter * 4, quarter * 4 + 4):
                        nc.tensor.matmul(
                            ps[:], lhsT=wq_t[kt][:, h * P:(h + 1) * P],
                            rhs=xc_tiles[(c, kt)][:],
                            start=(kt == 0), stop=(kt == NKT - 1))
                    if quarter == 3:
                        rope_evict(h, ps, c * 512)

                for h in range(NH):
                    fillers = [None] * 8
                    for q in range(4):
                        fillers[1 + 2 * q] = (lambda hh=h, qq=q: q3_unit(hh, qq))
                    attn_slot(0, h, fillers)

            # ---------------- phase 2: chunks 1-3 + o_proj ------------------
            wo_pool = ctx.enter_context(tc.tile_pool(name="wo", bufs=NDT))
            aT_pool = ctx.enter_context(tc.tile_pool(name="aT", bufs=1))
            osb_pool = ctx.enter_context(tc.tile_pool(name="osb", bufs=6))

            # aTbig[:, dtile*T + qt*128 : +128] = attn_t[qt][:, dtile].T
            aTbig = aT_pool.tile([P, NDT * T], bf16, tag="aT")
            wo_t = []
            for dtile in range(NDT):
                tl = wo_pool.tile([P, D_MODEL], bf16, tag="wo", name=f"wot{dtile}")
                wo_t.append(tl)
            for dtile in range(NDT):
                nc.gpsimd.dma_start(out=wo_t[dtile][:, 0:1024],
                                    in_=wo[dtile * P:(dtile + 1) * P, 0:1024])
            for dtile in range(NDT):
                nc.gpsimd.dma_start(out=wo_t[dtile][:, 1024:2048],
                                    in_=wo[dtile * P:(dtile + 1) * P, 1024:2048])

            def transp_unit(qcp, tt, pair):
                # transpose attn_t[qt] dtiles (2*pair, 2*pair+1) into aTbig,
                # via bf16 scratch bitcast-carved from a fresh op psum tile
                # (an op tile owns a full bank, so the transpose's start=True
                # bank-clear cannot disturb any in-flight accumulation).
                qt = qcp * 4 + tt
                scrt = op_tile()
                for s2 in range(2):
                    dtile = pair * 2 + s2
                    scr = scrt[:, s2 * SCRW:(s2 + 1) * SCRW].bitcast(bf16)
                    nc.tensor.transpose(
                        scr, attn_t[qt][:, dtile * P:(dtile + 1) * P], identity[:])
                    nc.vector.tensor_copy(
                        aTbig[:, dtile * T + qt * P:dtile * T + (qt + 1) * P], scr)

            op_state = {}

            def oproj_unit(tt, nchn, half):
                # half 0: dtiles 0-3 (start); half 1: dtiles 4-7 (stop+evict)
                if half == 0:
                    op_state[(tt, nchn)] = op_tile()
                ps = op_state[(tt, nchn)]
                for dtile in range(half * 4, half * 4 + 4):
                    nc.tensor.matmul(
                        ps[:],
                        lhsT=aTbig[:, dtile * T + tt * P:dtile * T + (tt + 1) * P],
                        rhs=wo_t[dtile][:, nchn * 512:(nchn + 1) * 512],
                        start=(dtile == 0), stop=(dtile == NDT - 1))
                if half == 1:
                    del op_state[(tt, nchn)]
                    osb = osb_pool.tile([P, 512], f32, tag="osb", name="osb")
                    nc.vector.tensor_copy(osb[:], ps[:])
                    # alternate output queues (sync is idle during phase 2)
                    eng = nc.gpsimd if (tt + nchn) % 2 == 0 else nc.sync
                    eng.dma_start(
                        out=out[tt * P:(tt + 1) * P, nchn * 512:(nchn + 1) * 512],
                        in_=osb[:])

            # filler scheduling: per chunk qc (1..3), slots h=0..7 carry
            # transposes of chunk qc-1 (slot h<4 -> tt=h, 4 pair-units at
            # points 4-7) and o_proj groups of chunk qc-1 (2 units each) from
            # a readiness queue.
            ready_groups = []   # (tt_glob, nchn) ready once tt transposed

            for qc in range(1, NCH):
                qcp = qc - 1
                for h in range(NH):
                    fillers = []
                    trans = []
                    if h < 4:
                        trans = [(lambda t=h, p=p2: transp_unit(qcp, t, p))
                                 for p2 in range(4)]
                    n_op = min(8 - len(trans), 4)
                    opu = []
                    while ready_groups and len(opu) + 2 <= n_op:
                        ttg, nchn = ready_groups.pop(0)
                        opu.append(lambda a=ttg, b=nchn: oproj_unit(a, b, 0))
                        opu.append(lambda a=ttg, b=nchn: oproj_unit(a, b, 1))
                    # op units first (points 0..), transposes at the tail
                    fillers = opu + trans
                    attn_slot(qc, h, fillers)
                    if h < 4:
                        ttg = qcp * 4 + h
                        for nchn in range(NCH):
                            ready_groups.append((ttg, nchn))

            # ---------------- tail: last normalize + transposes + o_proj ----
            pq, ph = st["pvs_hq"]
            pvs_last = st["pvs"]
            normalize(pq, ph, pvs_last)
            st["pvs"] = None
            qcp = NCH - 1
            for tt in range(4):
                for p2 in range(4):
                    transp_unit(qcp, tt, p2)
                for nchn in range(NCH):
                    ready_groups.append((qcp * 4 + tt, nchn))
            for ttg, nchn in ready_groups:
                oproj_unit(ttg, nchn, 0)
                oproj_unit(ttg, nchn, 1)

    nc.compile()
    return nc


def make_tables():
    inv_freq = 1.0 / (THETA ** (np.arange(0, HEAD_DIM, 2, dtype=np.float32)
                                / HEAD_DIM))          # [64]
    ang = np.arange(T, dtype=np.float32)[:, None] * inv_freq[None, :]  # [T, 64]
    cos = np.cos(ang).T.astype(np.float32)            # [64, T]
    sin = np.sin(ang).T.astype(np.float32)
    cos2 = np.concatenate([cos, cos], axis=0)         # [128, T]
    sinA = np.concatenate([-sin, sin], axis=0)        # [128, T]
    return np.ascontiguousarray(cos2), np.ascontiguousarray(sinA)


def make_in_maps(x, W_qkv, W_o, padding_mask, masked):
    cos2_v, sinA_v = make_tables()
    in_maps = []
    for c in range(N_CORES):
        b, half = c // 2, c % 2
        q0 = half * QDIM
        k0 = NUM_HEADS * HEAD_DIM + half * GDIM
        v0 = NUM_HEADS * HEAD_DIM + QUERY_GROUPS * HEAD_DIM + half * GDIM
        wvk_v = np.concatenate(
            [W_qkv[:, v0:v0 + GDIM], W_qkv[:, k0:k0 + GDIM]], axis=1)
        m = {
            "xt": np.ascontiguousarray(x[b].T).astype(BF16),
            "wq": np.ascontiguousarray(W_qkv[:, q0:q0 + QDIM]).astype(BF16),
            "wvk": np.ascontiguousarray(wvk_v).astype(BF16),
            "wo": np.ascontiguousarray(W_o[half * QDIM:(half + 1) * QDIM, :]).astype(BF16),
            "cos2": cos2_v, "sinA": sinA_v,
        }
        if masked:
            bias = np.where(padding_mask[b], 0.0, -1e30).astype(np.float32)  # [T]
            m["maskcol"] = np.ascontiguousarray(
                bias.reshape(NTT, P).T).astype(np.float32)
        in_maps.append(m)
    return in_maps


_nc_cache = {}


def kernel(x, W_qkv, W_o, padding_mask, trace=False):
    from concourse.bass_utils import run_bass_kernel_spmd

    x = np.asarray(x)
    W_qkv = np.asarray(W_qkv)
    W_o = np.asarray(W_o)
    padding_mask = np.asarray(padding_mask)
    masked = not bool(padding_mask.all())

    if masked not in _nc_cache:
        _nc_cache[masked] = build_nc(masked)
    nc = _nc_cache[masked]

    in_maps = make_in_maps(x, W_qkv, W_o, padding_mask, masked)
    res = run_bass_kernel_spmd(
        nc, in_maps, core_ids=list(range(N_CORES)),
        trace=trace, trace_cores=[0] if trace else None)

    out = np.empty((B, T, D_MODEL), np.float32)
    for b in range(B):
        out[b] = res.results[2 * b]["out"] + res.results[2 * b + 1]["out"]
    kernel.last_exec_time_ns = res.exec_time_ns
    kernel.last_results = res
    return out
